# revision 42
# baseline (speedup 1.0000x reference)
"""Trainium2 Bass kernel for AttentionDownsampler (nn_AttentionDownsampler_10264971837445).

Math (per batch b):
  patches[b, Y, X, p=(y,xi), c] = hr[b, c, 14Y+y, 14X+xi]
  logits[b, Y, X, p] = sum_c patches * w[c] + ab
  l2 = logits * mask[b, Y, X] * wkk[p] + bkk[p]
  attn = softmax_p(l2)
  out[b, c, Y, X] = sum_p patches[..., p, c] * attn[p]

Sharding: 8 cores = 4 batches x 2 halves of the H(=Y) axis; per-core shard is
8 rows x 16 X patches of 196 px x 384 c.

All bulk compute runs on the PE (2.4 GHz, 1 col/cyc bf16); DVE/ACT only do
small softmax tiles. The data is sent twice in bf16 (DVE has no fast reduce:
tensor_reduce/AMR run 1 col/cyc at 0.96 GHz, so any vector-engine reduction
of the 9.6M-element shard costs ~100us):

  - c-major copy [384c, row, X, 196p] feeds the scoring matmuls (contraction
    over c on partitions): 48 one-hot matmuls per row-pair -> PSUM
    lg2[16X, 392] as in the previous kernel.
  - p-major copy [98p, 2ph, row, X, 385] feeds the reduction (contraction
    over p on partitions). Column 384 is a constant 1.0 -> esum arrives in
    the same PSUM tile for free.

Softmax: t2 = lg2*mw2 + lkk2 (DVE; e196 factor folded in log space), ex =
Exp(t2) -> bf16 (ACT). Per row: PE-transpose ex[16, 98] -> [98, 16] per
p-half, DVE-copy the 16 columns onto the stride-17 diagonal of a zeroed
one-hot tile oh[98, 16X*16m] (off-diagonal stays zero forever; 2-deep parity
buffering), then 32 matmuls (16 X x 2 ph) with lhsT = oh[:, X, :] accumulate
psum[16X, 385]: row X only receives attn_X * D_X (other columns of oh are
zero), col 384 = esum. Normalize: reciprocal + tensor_scalar_mul -> SBUF,
one output DMA [16, 8, 384] fp32 at the end (host transposes back).

Budget (cost model): DMA 2x19.3MB = 107us @360GB/s (span driver), PE ~75us
(31 scoring + 41 reduction + transposes), DVE ~10us, ACT ~3us, GpSimd 0.
The previous kernel ran the reduction on DVE/GpSimd (~200us of engine work,
attn DRAM-bounce broadcast): 186-198us measured. This one targets ~115us.
"""

import sys

for _p in ("/opt/trn_rl_repo", "/root/.axon_site/_ro/trn_rl_repo"):
    if _p not in sys.path:
        sys.path.append(_p)

import ml_dtypes
import numpy as np

import concourse.bacc as bacc
import concourse.bass as bass_mod
import concourse.mybir as mybir
import concourse.tile as tile
from concourse.bass_utils import run_bass_kernel_spmd

K = 14          # patch size
C = 384         # channels
CCH = 128       # channel chunk (partitions)
NCH = C // CCH  # 3 chunks
NX = 16         # patches across W
P = K * K       # 196 pixels per patch
PH = P // 2     # 98: pixels per p-half (reduction contraction tile)
W2 = 2 * P      # 392 columns: a row-pair in one scoring matmul group
NC1 = C + 1     # 385: reduction rhs columns (384 c + ones column)
NCORES = 8
NROW = 8
NPAIRS = NROW // 2
VROWS = 2       # rows reduced on DVE/GpSimd from the c-copy (no p-copy sent)
PROWS = NROW - VROWS

FP32 = mybir.dt.float32
BF16 = mybir.dt.bfloat16


def build_nc(nrow=NROW):
    """Build the SPMD Bass program (identical on all cores)."""
    nc = bacc.Bacc("TRN2", target_bir_lowering=False, debug=False,
                   num_devices=NCORES)

    # c-major shard: [c, row, X, p]
    hr = nc.dram_tensor("hr", [C, nrow, NX, P], BF16, kind="ExternalInput")
    # p-major shard, rows VROWS..7 only: [p%98, ph, prow, X, 384c + ones]
    hrt = nc.dram_tensor("hrt", [PH, 2, PROWS, NX, NC1], BF16,
                         kind="ExternalInput")
    # one-hot scorer weights: woh[c, X, m] = w[c] if m == X else 0
    woh = nc.dram_tensor("woh", [C, NX, NX], BF16, kind="ExternalInput")
    # mw2[m, pair, ri*196+p] = mask[2*pair+ri, m] * wkk[p]
    mw2 = nc.dram_tensor("mw2", [NX, NPAIRS, W2], BF16, kind="ExternalInput")
    # lkk2[m, pair, ri*196+p] = ab*mask[2*pair+ri, m]*wkk[p] + bkk[p]
    lkk2 = nc.dram_tensor("lkk2", [NX, NPAIRS, W2], BF16, kind="ExternalInput")
    # identity for PE transpose
    ident = nc.dram_tensor("ident", [NX, NX], BF16, kind="ExternalInput")
    # out_t[X, prow, c] for PE-reduced rows (host transposes back)
    out = nc.dram_tensor("out", [NX, PROWS, C], FP32, kind="ExternalOutput")
    # vector-row output [c, vrow, X] and attn broadcast bounce scratch
    out2 = nc.dram_tensor("out2", [C, VROWS, NX], FP32, kind="ExternalOutput")
    attn_dram = nc.dram_tensor("attn_scratch", [NX, VROWS, P], BF16)

    with tile.TileContext(nc) as tc:
        _emit(tc, nc, nrow, hr, hrt, woh, mw2, lkk2, ident, out, out2,
              attn_dram)
    nc.finalize()
    return nc


def _emit(tc, nc, nrow, hr, hrt, woh, mw2, lkk2, ident, out, out2, attn_dram):
    import contextlib

    ctx = contextlib.ExitStack()
    with ctx:
        singles = ctx.enter_context(tc.tile_pool(name="singles", bufs=1))
        cdata0 = ctx.enter_context(tc.tile_pool(name="cdata0", bufs=3))
        cdata = ctx.enter_context(tc.tile_pool(name="cdata", bufs=4))
        pdata = ctx.enter_context(tc.tile_pool(name="pdata", bufs=7))
        orow_pool = ctx.enter_context(tc.tile_pool(name="orow", bufs=2))
        attnb_pool = ctx.enter_context(tc.tile_pool(name="attnb", bufs=1))
        small = ctx.enter_context(tc.tile_pool(name="small", bufs=2))
        psum_lg = ctx.enter_context(
            tc.tile_pool(name="psum_lg", bufs=2, space="PSUM"))
        psum_r = ctx.enter_context(
            tc.tile_pool(name="psum_r", bufs=2, space="PSUM"))
        psum_t = ctx.enter_context(
            tc.tile_pool(name="psum_t", bufs=4, space="PSUM"))

        # ---- constants (loaded once, small; off the sync queue so the
        # first data tile transfer starts immediately) ----
        woh_sb = singles.tile([CCH, NCH, NX, NX], BF16)
        for k in range(NCH):
            nc.scalar.dma_start(out=woh_sb[:, k, :, :],
                             in_=woh[k * CCH:(k + 1) * CCH, :, :])
        mw2_sb = singles.tile([NX, NPAIRS, W2], BF16)
        lkk2_sb = singles.tile([NX, NPAIRS, W2], BF16)
        ident_sb = singles.tile([NX, NX], BF16)
        nc.scalar.dma_start(out=mw2_sb, in_=mw2[:, :, :])
        nc.scalar.dma_start(out=lkk2_sb, in_=lkk2[:, :, :])
        nc.scalar.dma_start(out=ident_sb, in_=ident[:, :])

        # one-hot attn tiles [98, 16X * 16m], diagonal (stride 17) rewritten
        # per row, zeros elsewhere written once. 4-deep parity buffering
        # (rows r and r+2 in flight while r-2's matmuls may still be queued).
        oh = [[singles.tile([PH, NX * NX], BF16, name=f"oh{par}{ph}",
                            tag=f"oh{par}{ph}") for ph in range(2)]
              for par in range(4)]
        for par in range(4):
            for ph in range(2):
                nc.vector.memset(oh[par][ph], 0.0)

        # output accumulators: PE rows [16X, prow, 384c], vector rows
        # [128c, chunk, vrow, 16X]
        osb2 = singles.tile([CCH, NCH, VROWS, NX], FP32, name="osb2",
                            tag="osb2")
        attn_t = singles.tile([NX, VROWS, P], BF16, name="attn_t",
                              tag="attn_t")

        def transposes(pr, ex):
            # PE-transpose ex rows -> [98, 16] psum, DVE-scatter onto the
            # stride-17 diagonal of the one-hot tiles.
            for ri in range(2):
                par = (2 * pr + ri - VROWS) % 4
                for ph in range(2):
                    tp = psum_t.tile([PH, NX], BF16, tag="tp")
                    nc.tensor.transpose(
                        tp[:, :],
                        ex[:, ri * P + ph * PH:ri * P + (ph + 1) * PH],
                        ident_sb[:, :])
                    dst = oh[par][ph][:, :]
                    diag = bass_mod.AP(tensor=dst.tensor, offset=dst.offset,
                                       ap=[dst.ap[0], [NX + 1, NX]])
                    nc.vector.tensor_copy(diag, tp[:, :])

        def reductions(pr, pt):
            # 32 matmuls per row accumulate [16X, 385]; col 384 = esum.
            # ph-outer so the ph0 half runs while ph1's DMA is in flight.
            for ri in range(2):
                r = 2 * pr + ri
                par = (r - VROWS) % 4
                pr_ps = psum_r.tile([NX, NC1], FP32, tag="pr")
                for ph in range(2):
                    for X in range(NX):
                        nc.tensor.matmul(
                            pr_ps[:, :],
                            oh[par][ph][:, X * NX:(X + 1) * NX],
                            pt[2 * ri + ph][:, X, :],
                            start=(X == 0 and ph == 0),
                            stop=(X == NX - 1 and ph == 1),
                        )
                rcp = small.tile([NX, 1], FP32, tag="rcp")
                nc.vector.reciprocal(rcp, pr_ps[:, C:NC1])
                orow = orow_pool.tile([NX, C], FP32, tag="orow")
                nc.vector.tensor_scalar_mul(orow, pr_ps[:, 0:C], rcp)
                nc.scalar.dma_start(out=out[:, r - VROWS, :], in_=orow)

        scratch2 = singles.tile([CCH, P], BF16, name="scratch2",
                                tag="scratch2")

        def vrow_prep(ex0):
            # normalize attn for both vector rows and broadcast across
            # partitions via one DRAM bounce (write on the scalar HWDGE
            # queue, stride-0 partition read on the gpsimd SWDGE queue).
            for ri in range(VROWS):
                esum = small.tile([NX, 1], FP32, tag="esum")
                nc.vector.tensor_reduce(esum, ex0[:, ri * P:(ri + 1) * P],
                                        axis=mybir.AxisListType.X,
                                        op=mybir.AluOpType.add)
                rcpv = small.tile([NX, 1], FP32, tag="rcpv")
                nc.vector.reciprocal(rcpv, esum)
                nc.vector.tensor_scalar_mul(attn_t[:, ri, :],
                                            ex0[:, ri * P:(ri + 1) * P],
                                            rcpv)
            nc.gpsimd.dma_start(out=attn_dram[:, :, :], in_=attn_t)
            attnB = attnb_pool.tile([CCH, VROWS, NX, P], BF16, tag="attnB")
            for ri in range(VROWS):
                _src = attn_dram[:, ri, :]
                _bsrc = bass_mod.AP(tensor=_src.tensor, offset=_src.offset,
                                    ap=[[0, CCH], *_src.ap])
                nc.gpsimd.dma_start(out=attnB[:, ri, :, :], in_=_bsrc)
            return attnB

        def vrow_piece_dve(ri, k, dk0, attnB):
            # one c-chunk of a vector row, all on DVE (mult 2x bf16 + reduce)
            # the mult is in place: the pair-0 c-tile data is dead after it
            prod = dk0[k][:, ri, :, :]
            nc.vector.tensor_mul(prod, prod, attnB[:, ri, :, :])
            nc.vector.tensor_reduce(osb2[:, k, ri, :], prod,
                                    axis=mybir.AxisListType.X,
                                    op=mybir.AluOpType.add)

        def vrow_piece_gps_act(ri, k, dk0, attnB):
            # one c-chunk of a vector row: in-place GpSimd mult, ACT reduce
            prod = dk0[k][:, ri, :, :]
            nc.gpsimd.tensor_mul(prod, prod, attnB[:, ri, :, :])
            for X in range(NX):
                nc.scalar.activation(
                    scratch2, prod[:, X, :],
                    mybir.ActivationFunctionType.Copy,
                    accum_out=osb2[:, k, ri, X:X + 1])

        dk0 = None
        ex0 = None
        for pr in range(NPAIRS):
            # ---- DMA in exact need order: 3 c-tiles then p half-row tiles
            # (pair 0 has no p-copy; its rows run on the vector engines) ----
            dk = []
            for k in range(NCH):
                pool = cdata0 if pr == 0 else cdata
                t = pool.tile([CCH, 2, NX, P], BF16, tag="cdata")
                nc.sync.dma_start(
                    out=t, in_=hr[k * CCH:(k + 1) * CCH, 2 * pr:2 * pr + 2,
                                  :, :])
                dk.append(t)
            pt = []
            if pr > 0:
                for ri in range(2):
                    for ph in range(2):
                        t = pdata.tile([PH, NX, NC1], BF16, tag="pdata")
                        nc.sync.dma_start(
                            out=t,
                            in_=hrt[:, ph, 2 * pr + ri - VROWS, :, :])
                        pt.append(t)

            # ---- scoring: 48 one-hot matmuls (N=392), one accum group,
            # k-outer so PE starts as soon as chunk 0's DMA lands ----
            lg2 = psum_lg.tile([NX, W2], FP32, tag="lg")
            for k in range(NCH):
                for X in range(NX):
                    nc.tensor.matmul(
                        lg2[:, :],
                        woh_sb[:, k, X, :],
                        dk[k][:, :, X, :],
                        start=(k == 0 and X == 0),
                        stop=(k == NCH - 1 and X == NX - 1),
                    )

            # ---- softmax numerator: ex = exp(lg2*mw2 + lkk2) -> bf16 ----
            t2 = small.tile([NX, W2], FP32, tag="t2")
            nc.vector.tensor_mul(t2, lg2[:, :], mw2_sb[:, pr, :])
            nc.vector.tensor_add(t2, t2, lkk2_sb[:, pr, :])
            ex = small.tile([NX, W2], BF16, tag="ex")
            nc.scalar.activation(ex, t2, mybir.ActivationFunctionType.Exp)

            if pr == 0:
                # start the attn broadcast bounce early (the read queues
                # behind the input stream on the DMA engines, ~15us), but
                # emit NO dependent compute yet — it would head-of-line
                # block the DVE queue
                dk0, ex0 = dk, ex
                attnB0 = vrow_prep(ex0)
            else:
                # ---- reduce THIS pair immediately: p-tile slots must free
                # as fast as possible (DMA is the span driver; the PE has
                # slack to absorb the softmax-chain stall). Vector-row
                # chunks are spread across pairs, after each pair's softmax
                # and norm work, so they never block the critical DVE ops ----
                transposes(pr, ex)
                reductions(pr, pt)
                # gate the broadcast-dependent vector-row work to late sim
                # timestamps: the tile scheduler otherwise hoists it ahead of
                # the critical softmax/transpose-copy ops on DVE (its cost
                # model underestimates how long the attn broadcast queues
                # behind the input stream on the DMA engines)
                with tc.tile_wait_until(0.050 + 0.022 * (pr - 1)):
                    vrow_piece_dve(0, pr - 1, dk0, attnB0)
                if pr == 2:
                    for k in range(NCH):
                        with tc.tile_wait_until(0.062 + 0.013 * k):
                            vrow_piece_gps_act(1, k, dk0, attnB0)

        for k in range(NCH):
            nc.scalar.dma_start(out=out2[k * CCH:(k + 1) * CCH, :, :],
                                in_=osb2[:, k, :, :])


_NC_CACHE = {}


def _get_nc(nrow=NROW):
    if nrow not in _NC_CACHE:
        _NC_CACHE[nrow] = build_nc(nrow)
    return _NC_CACHE[nrow]


def regroup_shard(hr_slice):
    """[384, 112, 224] -> patch-grouped fp32 [384, 8, 16, 196]."""
    c, h, w = hr_slice.shape
    g = hr_slice.reshape(c, h // K, K, w // K, K).transpose(0, 1, 3, 2, 4)
    return np.ascontiguousarray(g.reshape(c, h // K, w // K, P))


def make_in_maps(hr_feats, guidance, attn_w, attn_b, w_kk, b_kk, dropout_mask,
                 nrow=NROW):
    b = hr_feats.shape[0]
    w = np.asarray(attn_w, np.float32)[0]                      # [384]
    ab = np.float32(np.asarray(attn_b)[0])
    wkk_flat = np.asarray(w_kk, np.float32).reshape(-1)        # [196]
    bkk_flat = np.asarray(b_kk, np.float32).reshape(-1)        # [196]
    mask = np.asarray(dropout_mask).astype(np.float32)[..., 0]  # [b, H, W]

    woh = np.zeros((C, NX, NX), np.float32)
    woh[:, np.arange(NX), np.arange(NX)] = w[:, None]          # [c, X, m]
    woh = woh.astype(ml_dtypes.bfloat16)
    ident = np.eye(NX, dtype=ml_dtypes.bfloat16)

    in_maps = []
    for core in range(NCORES):
        bi, half = divmod(core, 2)
        bi = bi % b
        hrg = regroup_shard(
            np.asarray(hr_feats[bi, :, 112 * half:112 * half + K * nrow, :],
                       np.float32))                            # [384, 8, 16, 196] f32
        hrc = hrg.astype(ml_dtypes.bfloat16)                   # c-major copy
        # p-major copy, rows VROWS..7: [98, 2ph, prow, X, 385] + ones column
        hpr = hrg[:, VROWS:, :, :]                             # [384, 6, 16, 196]
        hp = hpr.transpose(3, 1, 2, 0).reshape(2, PH, PROWS, NX, C)
        hp = hp.transpose(1, 0, 2, 3, 4)                       # [98, 2, 6, 16, 384]
        hrt = np.empty((PH, 2, PROWS, NX, NC1), ml_dtypes.bfloat16)
        hrt[..., 0:C] = hp.astype(ml_dtypes.bfloat16)
        hrt[..., C] = np.float32(1.0)
        mrow = mask[bi, 8 * half:8 * half + nrow, :]           # [nrow, 16]
        mcol = np.ascontiguousarray(mrow.T)                    # [16(X), nrow]
        # mw2[m, pair, ri*196+p] = mask[2*pair+ri, m] * wkk[p]
        mw2 = (mcol[:, :, None] * wkk_flat[None, None, :])     # [16, nrow, 196]
        lkk2 = ab * mw2 + bkk_flat[None, None, :]
        mw2 = np.ascontiguousarray(
            mw2.reshape(NX, NPAIRS, W2)).astype(ml_dtypes.bfloat16)
        lkk2 = np.ascontiguousarray(
            lkk2.reshape(NX, NPAIRS, W2)).astype(ml_dtypes.bfloat16)
        in_maps.append({
            "hr": hrc, "hrt": hrt, "woh": woh, "mw2": mw2, "lkk2": lkk2,
            "ident": ident,
        })
    return in_maps


def kernel(hr_feats, guidance, attn_w, attn_b, w_kk, b_kk, dropout_mask,
           trace=False):
    hr_feats = np.asarray(hr_feats, np.float32)
    b, c, h, wimg = hr_feats.shape
    H = h // K
    nc = _get_nc(NROW)
    in_maps = make_in_maps(hr_feats, guidance, attn_w, attn_b, w_kk, b_kk,
                           dropout_mask)
    res = run_bass_kernel_spmd(nc, in_maps, core_ids=list(range(NCORES)),
                               trace=trace)
    full = np.empty((b, C, H, NX), np.float32)
    for core in range(NCORES):
        bi, half = divmod(core, 2)
        r0 = 8 * half
        # vector rows: out2[c, vrow, X]
        full[bi, :, r0:r0 + VROWS, :] = res.results[core]["out2"]
        # PE rows: out_t[X, prow, c] -> [c, prow, X]
        full[bi, :, r0 + VROWS:r0 + 8, :] = \
            res.results[core]["out"].transpose(2, 1, 0)
    if trace:
        return full, res
    return full


# revision 43
# speedup vs baseline: 1.0188x; 1.0188x over previous
"""Trainium2 Bass kernel for AttentionDownsampler (nn_AttentionDownsampler_10264971837445).

Math (per batch b):
  patches[b, Y, X, p=(y,xi), c] = hr[b, c, 14Y+y, 14X+xi]
  logits[b, Y, X, p] = sum_c patches * w[c] + ab
  l2 = logits * mask[b, Y, X] * wkk[p] + bkk[p]
  attn = softmax_p(l2)
  out[b, c, Y, X] = sum_p patches[..., p, c] * attn[p]

Sharding: 8 cores = 4 batches x 2 halves of the H(=Y) axis; per-core shard is
8 rows x 16 X patches of 196 px x 384 c.

The kernel is DMA-bound: the per-core DMA fabric sustains ~300 GB/s (16
engines x ~19 GB/s; 512B bus packets), so span ~= bytes streamed. The bulk
compute runs on the PE (2.4 GHz, 1 col/cyc bf16). DVE has no fast reduce
(tensor_reduce/AMR run 1 col/cyc at 0.96 GHz, ~100us for a full-shard
reduction), so the softmax-weighted reduction needs the PE too - which needs
the data twice (PE contracts only over the partition axis):

  - c-major copy [384c, row, X, 196p] bf16 (19.3MB) feeds the scoring
    matmuls: 48 one-hot matmuls per row-pair -> PSUM lg2[16X, 392].
  - p-major copy [98p, 2ph, prow, X, 384c+ones] bf16, ROWS 2..7 ONLY
    (14.5MB), feeds the PE reduction (contraction over p on partitions).
    The ones column makes esum land in the same PSUM tile for free.
  - rows 0-1 are instead reduced from the (already needed) c-major copy on
    the vector engines (in-place mult on DVE / GpSimd + 3D tensor_reduce on
    DVE / ACT accum), with attn broadcast across partitions via a DRAM
    bounce. Net: -3.2MB of stream for ~35us of spare vector-engine time.
    (VROWS=4 measured identical to VROWS=2: the PE reduces a row ~4x more
    efficiently, so trading more stream for vector work is a wash.)

Softmax: t2 = lg2*mw2 + lkk2 (DVE, bf16 consts; the exp(ab*mask*wkk+bkk)
factor is folded in log space), ex = Exp(t2) -> bf16 (ACT). Per PE row:
PE-transpose ex[16, 98] -> [98, 16] per p-half, DVE-copy onto the stride-17
diagonal of a zeroed one-hot tile oh[98, 16X*16m] (off-diagonal stays zero
forever), then 32 matmuls with lhsT = oh[:, X, :] accumulate psum[16X, 385]:
row X only receives attn_X * D_X, col 384 = esum. Normalize: reciprocal +
tensor_scalar_mul, per-row output DMA (host transposes back).

Scheduling (everything follows from DMA being the bottleneck):
  - one sync-queue DMA stream in exact need order [c x3, p-half x4] per
    pair; consts/outputs on the scalar queue; the attn bounce write AND
    broadcast reads both on the gpsimd queue (in-order => race-free; a
    cross-queue write->read pair is NOT ordered by the framework).
  - reductions run in the same pair as their softmax: p-tile pool slots must
    recycle as fast as possible; the PE has slack to eat the softmax-chain
    stall (deep pools: cdata0 3, cdata 4, pdata 7 half-row tiles).
  - the vector-row chunks are gated via tc.tile_wait_until to late sim
    timestamps: the tile scheduler's cost model underestimates how long the
    attn broadcast queues behind the input stream (~15us), and otherwise
    hoists broadcast-dependent DVE work ahead of the critical softmax /
    transpose-copy ops (head-of-line blocking cost ~30us when it happens).

Measured: ~140-145us HW exec (was 186-198us for the all-vector-engine
baseline), rel err 7.7e-3. Floor: ~11.5us fixed preamble + ~120us stream
(34.4MB at ~93% DMA occupancy) + ~8us tail + ~11us fixed teardown.
"""

import sys

for _p in ("/opt/trn_rl_repo", "/root/.axon_site/_ro/trn_rl_repo"):
    if _p not in sys.path:
        sys.path.append(_p)

import ml_dtypes
import numpy as np

import concourse.bacc as bacc
import concourse.bass as bass_mod
import concourse.mybir as mybir
import concourse.tile as tile
from concourse.bass_utils import run_bass_kernel_spmd

K = 14          # patch size
C = 384         # channels
CCH = 128       # channel chunk (partitions)
NCH = C // CCH  # 3 chunks
NX = 16         # patches across W
P = K * K       # 196 pixels per patch
PH = P // 2     # 98: pixels per p-half (reduction contraction tile)
W2 = 2 * P      # 392 columns: a row-pair in one scoring matmul group
NC1 = C + 1     # 385: reduction rhs columns (384 c + ones column)
NCORES = 8
NROW = 8
NPAIRS = NROW // 2
VROWS = 2       # rows reduced on DVE/GpSimd from the c-copy (no p-copy sent)
PROWS = NROW - VROWS

FP32 = mybir.dt.float32
BF16 = mybir.dt.bfloat16


def build_nc(nrow=NROW):
    """Build the SPMD Bass program (identical on all cores)."""
    nc = bacc.Bacc("TRN2", target_bir_lowering=False, debug=False,
                   num_devices=NCORES)

    # c-major shard: [c, row, X, p]
    hr = nc.dram_tensor("hr", [C, nrow, NX, P], BF16, kind="ExternalInput")
    # p-major shard, rows VROWS..7 only: [p%98, ph, prow, X, 384c + ones]
    hrt = nc.dram_tensor("hrt", [PH, 2, PROWS, NX, NC1], BF16,
                         kind="ExternalInput")
    # one-hot scorer weights: woh[c, X, m] = w[c] if m == X else 0
    woh = nc.dram_tensor("woh", [C, NX, NX], BF16, kind="ExternalInput")
    # mw2[m, pair, ri*196+p] = mask[2*pair+ri, m] * wkk[p]
    mw2 = nc.dram_tensor("mw2", [NX, NPAIRS, W2], BF16, kind="ExternalInput")
    # lkk2[m, pair, ri*196+p] = ab*mask[2*pair+ri, m]*wkk[p] + bkk[p]
    lkk2 = nc.dram_tensor("lkk2", [NX, NPAIRS, W2], BF16, kind="ExternalInput")
    # identity for PE transpose
    ident = nc.dram_tensor("ident", [NX, NX], BF16, kind="ExternalInput")
    # out_t[X, prow, c] for PE-reduced rows (host transposes back)
    out = nc.dram_tensor("out", [NX, PROWS, C], FP32, kind="ExternalOutput")
    # vector-row output [c, vrow, X] and attn broadcast bounce scratch
    out2 = nc.dram_tensor("out2", [C, VROWS, NX], FP32, kind="ExternalOutput")
    attn_dram = nc.dram_tensor("attn_scratch", [NX, VROWS, P], BF16)

    with tile.TileContext(nc) as tc:
        _emit(tc, nc, nrow, hr, hrt, woh, mw2, lkk2, ident, out, out2,
              attn_dram)
    nc.finalize()
    return nc


def _emit(tc, nc, nrow, hr, hrt, woh, mw2, lkk2, ident, out, out2, attn_dram):
    import contextlib

    ctx = contextlib.ExitStack()
    with ctx:
        singles = ctx.enter_context(tc.tile_pool(name="singles", bufs=1))
        cdata0 = ctx.enter_context(tc.tile_pool(name="cdata0", bufs=3))
        cdata = ctx.enter_context(tc.tile_pool(name="cdata", bufs=4))
        pdata = ctx.enter_context(tc.tile_pool(name="pdata", bufs=7))
        orow_pool = ctx.enter_context(tc.tile_pool(name="orow", bufs=2))
        attnb_pool = ctx.enter_context(tc.tile_pool(name="attnb", bufs=1))
        small = ctx.enter_context(tc.tile_pool(name="small", bufs=2))
        psum_lg = ctx.enter_context(
            tc.tile_pool(name="psum_lg", bufs=2, space="PSUM"))
        psum_r = ctx.enter_context(
            tc.tile_pool(name="psum_r", bufs=2, space="PSUM"))
        psum_t = ctx.enter_context(
            tc.tile_pool(name="psum_t", bufs=4, space="PSUM"))

        # ---- constants (loaded once, small; off the sync queue so the
        # first data tile transfer starts immediately) ----
        woh_sb = singles.tile([CCH, NCH, NX, NX], BF16)
        for k in range(NCH):
            nc.scalar.dma_start(out=woh_sb[:, k, :, :],
                             in_=woh[k * CCH:(k + 1) * CCH, :, :])
        mw2_sb = singles.tile([NX, NPAIRS, W2], BF16)
        lkk2_sb = singles.tile([NX, NPAIRS, W2], BF16)
        ident_sb = singles.tile([NX, NX], BF16)
        nc.scalar.dma_start(out=mw2_sb, in_=mw2[:, :, :])
        nc.scalar.dma_start(out=lkk2_sb, in_=lkk2[:, :, :])
        nc.scalar.dma_start(out=ident_sb, in_=ident[:, :])

        # one-hot attn tiles [98, 16X * 16m], diagonal (stride 17) rewritten
        # per row, zeros elsewhere written once. 4-deep parity buffering
        # (rows r and r+2 in flight while r-2's matmuls may still be queued).
        oh = [[singles.tile([PH, NX * NX], BF16, name=f"oh{par}{ph}",
                            tag=f"oh{par}{ph}") for ph in range(2)]
              for par in range(4)]
        for par in range(4):
            for ph in range(2):
                nc.vector.memset(oh[par][ph], 0.0)

        # output accumulators: PE rows [16X, prow, 384c], vector rows
        # [128c, chunk, vrow, 16X]
        osb2 = singles.tile([CCH, NCH, VROWS, NX], FP32, name="osb2",
                            tag="osb2")
        attn_t = singles.tile([NX, VROWS, P], BF16, name="attn_t",
                              tag="attn_t")

        def transposes(pr, ex):
            # PE-transpose ex rows -> [98, 16] psum, DVE-scatter onto the
            # stride-17 diagonal of the one-hot tiles.
            for ri in range(2):
                par = (2 * pr + ri - VROWS) % 4
                for ph in range(2):
                    tp = psum_t.tile([PH, NX], BF16, tag="tp")
                    nc.tensor.transpose(
                        tp[:, :],
                        ex[:, ri * P + ph * PH:ri * P + (ph + 1) * PH],
                        ident_sb[:, :])
                    dst = oh[par][ph][:, :]
                    diag = bass_mod.AP(tensor=dst.tensor, offset=dst.offset,
                                       ap=[dst.ap[0], [NX + 1, NX]])
                    nc.vector.tensor_copy(diag, tp[:, :])

        def reductions(pr, pt):
            # 32 matmuls per row accumulate [16X, 385]; col 384 = esum.
            # ph-outer so the ph0 half runs while ph1's DMA is in flight.
            for ri in range(2):
                r = 2 * pr + ri
                par = (r - VROWS) % 4
                pr_ps = psum_r.tile([NX, NC1], FP32, tag="pr")
                for ph in range(2):
                    for X in range(NX):
                        nc.tensor.matmul(
                            pr_ps[:, :],
                            oh[par][ph][:, X * NX:(X + 1) * NX],
                            pt[2 * ri + ph][:, X, :],
                            start=(X == 0 and ph == 0),
                            stop=(X == NX - 1 and ph == 1),
                        )
                rcp = small.tile([NX, 1], FP32, tag="rcp")
                nc.vector.reciprocal(rcp, pr_ps[:, C:NC1])
                orow = orow_pool.tile([NX, C], FP32, tag="orow")
                nc.vector.tensor_scalar_mul(orow, pr_ps[:, 0:C], rcp)
                nc.scalar.dma_start(out=out[:, r - VROWS, :], in_=orow)

        scratch2 = singles.tile([CCH, P], BF16, name="scratch2",
                                tag="scratch2")

        def vrow_prep(ex0):
            # normalize attn for both vector rows and broadcast across
            # partitions via one DRAM bounce (write on the scalar HWDGE
            # queue, stride-0 partition read on the gpsimd SWDGE queue).
            for ri in range(VROWS):
                esum = small.tile([NX, 1], FP32, tag="esum")
                nc.vector.tensor_reduce(esum, ex0[:, ri * P:(ri + 1) * P],
                                        axis=mybir.AxisListType.X,
                                        op=mybir.AluOpType.add)
                rcpv = small.tile([NX, 1], FP32, tag="rcpv")
                nc.vector.reciprocal(rcpv, esum)
                nc.vector.tensor_scalar_mul(attn_t[:, ri, :],
                                            ex0[:, ri * P:(ri + 1) * P],
                                            rcpv)
            nc.gpsimd.dma_start(out=attn_dram[:, :, :], in_=attn_t)
            attnB = attnb_pool.tile([CCH, VROWS, NX, P], BF16, tag="attnB")
            for ri in range(VROWS):
                _src = attn_dram[:, ri, :]
                _bsrc = bass_mod.AP(tensor=_src.tensor, offset=_src.offset,
                                    ap=[[0, CCH], *_src.ap])
                nc.gpsimd.dma_start(out=attnB[:, ri, :, :], in_=_bsrc)
            return attnB

        def vrow_piece_dve(ri, k, dk0, attnB):
            # one c-chunk of a vector row, all on DVE (mult 2x bf16 + reduce)
            # the mult is in place: the pair-0 c-tile data is dead after it
            prod = dk0[k][:, ri, :, :]
            nc.vector.tensor_mul(prod, prod, attnB[:, ri, :, :])
            nc.vector.tensor_reduce(osb2[:, k, ri, :], prod,
                                    axis=mybir.AxisListType.X,
                                    op=mybir.AluOpType.add)

        def vrow_piece_gps_act(ri, k, dk0, attnB):
            # one c-chunk of a vector row: in-place GpSimd mult, ACT reduce
            prod = dk0[k][:, ri, :, :]
            nc.gpsimd.tensor_mul(prod, prod, attnB[:, ri, :, :])
            for X in range(NX):
                nc.scalar.activation(
                    scratch2, prod[:, X, :],
                    mybir.ActivationFunctionType.Copy,
                    accum_out=osb2[:, k, ri, X:X + 1])

        dk0 = None
        ex0 = None
        for pr in range(NPAIRS):
            # ---- DMA in exact need order: 3 c-tiles then p half-row tiles
            # (pair 0 has no p-copy; its rows run on the vector engines) ----
            dk = []
            for k in range(NCH):
                pool = cdata0 if pr == 0 else cdata
                t = pool.tile([CCH, 2, NX, P], BF16, tag="cdata")
                nc.sync.dma_start(
                    out=t, in_=hr[k * CCH:(k + 1) * CCH, 2 * pr:2 * pr + 2,
                                  :, :])
                dk.append(t)
            pt = []
            if pr > 0:
                for ri in range(2):
                    for ph in range(2):
                        t = pdata.tile([PH, NX, NC1], BF16, tag="pdata")
                        nc.sync.dma_start(
                            out=t,
                            in_=hrt[:, ph, 2 * pr + ri - VROWS, :, :])
                        pt.append(t)

            # ---- scoring: 48 one-hot matmuls (N=392), one accum group,
            # k-outer so PE starts as soon as chunk 0's DMA lands ----
            lg2 = psum_lg.tile([NX, W2], FP32, tag="lg")
            for k in range(NCH):
                for X in range(NX):
                    nc.tensor.matmul(
                        lg2[:, :],
                        woh_sb[:, k, X, :],
                        dk[k][:, :, X, :],
                        start=(k == 0 and X == 0),
                        stop=(k == NCH - 1 and X == NX - 1),
                    )

            # ---- softmax numerator: ex = exp(lg2*mw2 + lkk2) -> bf16 ----
            t2 = small.tile([NX, W2], FP32, tag="t2")
            nc.vector.tensor_mul(t2, lg2[:, :], mw2_sb[:, pr, :])
            nc.vector.tensor_add(t2, t2, lkk2_sb[:, pr, :])
            ex = small.tile([NX, W2], BF16, tag="ex")
            nc.scalar.activation(ex, t2, mybir.ActivationFunctionType.Exp)

            if pr == 0:
                # start the attn broadcast bounce early (the read queues
                # behind the input stream on the DMA engines, ~15us), but
                # emit NO dependent compute yet — it would head-of-line
                # block the DVE queue
                dk0, ex0 = dk, ex
                attnB0 = vrow_prep(ex0)
            else:
                # ---- reduce THIS pair immediately: p-tile slots must free
                # as fast as possible (DMA is the span driver; the PE has
                # slack to absorb the softmax-chain stall). Vector-row
                # chunks are spread across pairs, after each pair's softmax
                # and norm work, so they never block the critical DVE ops ----
                transposes(pr, ex)
                reductions(pr, pt)
                # gate the broadcast-dependent vector-row work to late sim
                # timestamps: the tile scheduler otherwise hoists it ahead of
                # the critical softmax/transpose-copy ops on DVE (its cost
                # model underestimates how long the attn broadcast queues
                # behind the input stream on the DMA engines)
                with tc.tile_wait_until(0.050 + 0.022 * (pr - 1)):
                    vrow_piece_dve(0, pr - 1, dk0, attnB0)
                if pr == 2:
                    for k in range(NCH):
                        with tc.tile_wait_until(0.062 + 0.013 * k):
                            vrow_piece_gps_act(1, k, dk0, attnB0)

        for k in range(NCH):
            nc.scalar.dma_start(out=out2[k * CCH:(k + 1) * CCH, :, :],
                                in_=osb2[:, k, :, :])


_NC_CACHE = {}


def _get_nc(nrow=NROW):
    if nrow not in _NC_CACHE:
        _NC_CACHE[nrow] = build_nc(nrow)
    return _NC_CACHE[nrow]


def regroup_shard(hr_slice):
    """[384, 112, 224] -> patch-grouped fp32 [384, 8, 16, 196]."""
    c, h, w = hr_slice.shape
    g = hr_slice.reshape(c, h // K, K, w // K, K).transpose(0, 1, 3, 2, 4)
    return np.ascontiguousarray(g.reshape(c, h // K, w // K, P))


def make_in_maps(hr_feats, guidance, attn_w, attn_b, w_kk, b_kk, dropout_mask,
                 nrow=NROW):
    b = hr_feats.shape[0]
    w = np.asarray(attn_w, np.float32)[0]                      # [384]
    ab = np.float32(np.asarray(attn_b)[0])
    wkk_flat = np.asarray(w_kk, np.float32).reshape(-1)        # [196]
    bkk_flat = np.asarray(b_kk, np.float32).reshape(-1)        # [196]
    mask = np.asarray(dropout_mask).astype(np.float32)[..., 0]  # [b, H, W]

    woh = np.zeros((C, NX, NX), np.float32)
    woh[:, np.arange(NX), np.arange(NX)] = w[:, None]          # [c, X, m]
    woh = woh.astype(ml_dtypes.bfloat16)
    ident = np.eye(NX, dtype=ml_dtypes.bfloat16)

    in_maps = []
    for core in range(NCORES):
        bi, half = divmod(core, 2)
        bi = bi % b
        hrg = regroup_shard(
            np.asarray(hr_feats[bi, :, 112 * half:112 * half + K * nrow, :],
                       np.float32))                            # [384, 8, 16, 196] f32
        hrc = hrg.astype(ml_dtypes.bfloat16)                   # c-major copy
        # p-major copy, rows VROWS..7: [98, 2ph, prow, X, 385] + ones column
        hpr = hrg[:, VROWS:, :, :]                             # [384, 6, 16, 196]
        hp = hpr.transpose(3, 1, 2, 0).reshape(2, PH, PROWS, NX, C)
        hp = hp.transpose(1, 0, 2, 3, 4)                       # [98, 2, 6, 16, 384]
        hrt = np.empty((PH, 2, PROWS, NX, NC1), ml_dtypes.bfloat16)
        hrt[..., 0:C] = hp.astype(ml_dtypes.bfloat16)
        hrt[..., C] = np.float32(1.0)
        mrow = mask[bi, 8 * half:8 * half + nrow, :]           # [nrow, 16]
        mcol = np.ascontiguousarray(mrow.T)                    # [16(X), nrow]
        # mw2[m, pair, ri*196+p] = mask[2*pair+ri, m] * wkk[p]
        mw2 = (mcol[:, :, None] * wkk_flat[None, None, :])     # [16, nrow, 196]
        lkk2 = ab * mw2 + bkk_flat[None, None, :]
        mw2 = np.ascontiguousarray(
            mw2.reshape(NX, NPAIRS, W2)).astype(ml_dtypes.bfloat16)
        lkk2 = np.ascontiguousarray(
            lkk2.reshape(NX, NPAIRS, W2)).astype(ml_dtypes.bfloat16)
        in_maps.append({
            "hr": hrc, "hrt": hrt, "woh": woh, "mw2": mw2, "lkk2": lkk2,
            "ident": ident,
        })
    return in_maps


def kernel(hr_feats, guidance, attn_w, attn_b, w_kk, b_kk, dropout_mask,
           trace=False):
    hr_feats = np.asarray(hr_feats, np.float32)
    b, c, h, wimg = hr_feats.shape
    H = h // K
    nc = _get_nc(NROW)
    in_maps = make_in_maps(hr_feats, guidance, attn_w, attn_b, w_kk, b_kk,
                           dropout_mask)
    res = run_bass_kernel_spmd(nc, in_maps, core_ids=list(range(NCORES)),
                               trace=trace)
    full = np.empty((b, C, H, NX), np.float32)
    for core in range(NCORES):
        bi, half = divmod(core, 2)
        r0 = 8 * half
        # vector rows: out2[c, vrow, X]
        full[bi, :, r0:r0 + VROWS, :] = res.results[core]["out2"]
        # PE rows: out_t[X, prow, c] -> [c, prow, X]
        full[bi, :, r0 + VROWS:r0 + 8, :] = \
            res.results[core]["out"].transpose(2, 1, 0)
    if trace:
        return full, res
    return full
